# revision 7
# baseline (speedup 1.0000x reference)
"""Trainium2 Bass kernel for nn_Mlp_StaticRoutedLoRAExpert.

Computation (per token chunk with static expert e):
    h = gelu(x @ w1.T + bias1 + SCALE * (x @ a1[e].T) @ b1[e].T)
    y = h @ w2.T + bias2 + SCALE * (h @ a2[e].T) @ b2[e].T

Key choices:
  * The device folds LoRA into per-expert dense weights on the fly:
    W1e = w1 + SCALE*b1[e]@a1[e]  (same for layer 2), one rank-64 matmul
    plus a DVE add per 128x512 weight slice.  Only the raw pieces
    (w1, w2, a*, b*; 11.4MB instead of 18.9MB folded) are shipped per
    core, which matters because input staging is a large part of the
    measured execution window.
  * Weights and activations are bf16 (PE runs 1 cycle/row for bf16, same
    as fp32r, but half the SBUF/DMA); accumulation stays fp32 in PSUM.
  * Folded weights live in SBUF for one expert *run* at a time (tiles
    are grouped per expert, so the fold runs once per expert run).
  * fc1+gelu+fc2 are fused per 512-token tile: hidden activations never
    leave SBUF.
  * Data-parallel over batch: 4 batch rows per core on 8 cores.
"""

import numpy as np
import ml_dtypes

BF16 = ml_dtypes.bfloat16
SCALE = 128.0 / 64.0
B, S, IN, HID, OUT, E, R = 32, 1280, 768, 3072, 768, 2, 64
NCORES = 8
BPC = B // NCORES          # batch rows per core
TPC = BPC * S              # tokens per core
P = 128
KI = IN // P               # 6  input k-chunks
KH = HID // P              # 24 hidden chunks
KO = OUT // P              # 6  output chunks
MAX_T = 512                # PSUM bank limit on the moving dim
FS = 512                   # fold slice width (HID direction)
FS2 = 384                  # fold slice width for layer 2 (OUT=768=2x384)

_nc_cache: dict = {}


def _plan_tiles(chunk_sizes, expert_ids):
    """Token layout per core: chunks grouped so each tile is one expert."""
    tiles = []
    groups = []
    col = 0
    s_start = 0
    for sz, e in zip(chunk_sizes, expert_ids):
        groups.append((s_start, sz))
        gsz = BPC * sz
        off = 0
        while off < gsz:
            t = min(MAX_T, gsz - off)
            tiles.append((col + off, t, int(e)))
            off += t
        col += gsz
        s_start += sz
    assert col == TPC
    return tuple(tiles), tuple(groups)


def _build(tiles):
    import concourse.bacc as bacc
    import concourse.mybir as mybir
    import concourse.tile as tile

    dt = mybir.dt
    f32 = dt.float32
    bf16 = dt.bfloat16
    AF = mybir.ActivationFunctionType

    nc = bacc.Bacc("TRN2", target_bir_lowering=False, num_devices=NCORES)

    xt_d = nc.dram_tensor("xt", [IN, TPC], bf16, kind="ExternalInput")
    w1t_d = nc.dram_tensor("w1t", [IN, HID], bf16, kind="ExternalInput")
    a1t_d = nc.dram_tensor("a1t", [E, R, IN], bf16, kind="ExternalInput")
    b1t_d = nc.dram_tensor("b1t", [E, R, HID], bf16, kind="ExternalInput")
    b1v_d = nc.dram_tensor("bias1", [HID], f32, kind="ExternalInput")
    w2t_d = nc.dram_tensor("w2t", [HID, OUT], bf16, kind="ExternalInput")
    a2t_d = nc.dram_tensor("a2t", [E, R, HID], bf16, kind="ExternalInput")
    b2t_d = nc.dram_tensor("b2t", [E, R, OUT], bf16, kind="ExternalInput")
    b2v_d = nc.dram_tensor("bias2", [OUT], f32, kind="ExternalInput")
    yt_d = nc.dram_tensor("yt", [OUT, TPC], bf16, kind="ExternalOutput")

    # contiguous expert runs
    runs = []
    for (col, T, e) in tiles:
        if runs and runs[-1][0] == e:
            runs[-1][1].append((col, T))
        else:
            runs.append((e, [(col, T)]))

    def load_x(xpool, col, T):
        t = xpool.tile([P, KI, T], bf16, name="xio", tag="xio")
        nc.sync.dma_start(
            t[:], xt_d[:, col:col + T].rearrange("(k p) c -> p k c", p=P)
        )
        return t

    with tile.TileContext(nc) as tc:
        with (
            tc.tile_pool(name="const", bufs=1) as cpool,
            tc.tile_pool(name="wc", bufs=1) as wpool,
            tc.tile_pool(name="d1", bufs=4) as dpool1,
            tc.tile_pool(name="d2", bufs=12) as dpool2,
            tc.tile_pool(name="xp", bufs=3) as xpool,
            tc.tile_pool(name="hp", bufs=26) as hpool,
            tc.tile_pool(name="yp", bufs=3) as ypool,
            tc.tile_pool(name="hps", bufs=4, space="PSUM") as hps_pool,
            tc.tile_pool(name="yps", bufs=2, space="PSUM") as yps_pool,
            tc.tile_pool(name="fps", bufs=2, space="PSUM") as fps_pool,
        ):
            # first x tile first so the PE can start as soon as possible
            xio = {0: load_x(xpool, tiles[0][0], tiles[0][1])}

            # LoRA factors, all experts resident (tiny)
            a1_s = cpool.tile([R, E, IN], bf16)
            nc.sync.dma_start(a1_s[:], a1t_d.ap().rearrange("e r i -> r e i"))
            b1_s = cpool.tile([R, E, HID], bf16)
            nc.sync.dma_start(b1_s[:], b1t_d.ap().rearrange("e r h -> r e h"))
            bias1_s = cpool.tile([P, KH], f32)
            nc.sync.dma_start(bias1_s[:], b1v_d.ap().rearrange("(c p) -> p c", p=P))
            a2_s = cpool.tile([R, E, HID], bf16)
            nc.sync.dma_start(a2_s[:], a2t_d.ap().rearrange("e r h -> r e h"))
            b2_s = cpool.tile([R, E, OUT], bf16)
            nc.sync.dma_start(b2_s[:], b2t_d.ap().rearrange("e r o -> r e o"))
            bias2_s = cpool.tile([P, KO], f32)
            nc.sync.dma_start(bias2_s[:], b2v_d.ap().rearrange("(c p) -> p c", p=P))

            tile_idx = 0
            for (e, run_tiles) in runs:
                # ---- fold weights for this expert into SBUF ----
                w1_s = {}
                for k in range(KI):
                    dense = dpool1.tile([P, HID], bf16, name="d1", tag="d1")
                    nc.sync.dma_start(dense[:], w1t_d[k * P:(k + 1) * P, :])
                    wt = wpool.tile([P, HID], bf16, name=f"w1_{k}", tag=f"w1_{k}")
                    for s in range(HID // FS):
                        ps = fps_pool.tile([P, FS], f32, name="fps", tag="fps")
                        nc.tensor.matmul(
                            ps[:],
                            a1_s[:, e, k * P:(k + 1) * P],
                            b1_s[:, e, s * FS:(s + 1) * FS],
                            start=True, stop=True,
                        )
                        nc.vector.tensor_add(
                            wt[:, s * FS:(s + 1) * FS],
                            dense[:, s * FS:(s + 1) * FS],
                            ps[:],
                        )
                    w1_s[k] = wt
                w2_s = {}
                for m in range(KH):
                    dense = dpool2.tile([P, OUT], bf16, name="d2", tag="d2")
                    nc.sync.dma_start(dense[:], w2t_d[m * P:(m + 1) * P, :])
                    wt = wpool.tile([P, OUT], bf16, name=f"w2_{m}", tag=f"w2_{m}")
                    for s in range(OUT // FS2):
                        ps = fps_pool.tile([P, FS2], f32, name="fps", tag="fps")
                        nc.tensor.matmul(
                            ps[:],
                            a2_s[:, e, m * P:(m + 1) * P],
                            b2_s[:, e, s * FS2:(s + 1) * FS2],
                            start=True, stop=True,
                        )
                        nc.vector.tensor_add(
                            wt[:, s * FS2:(s + 1) * FS2],
                            dense[:, s * FS2:(s + 1) * FS2],
                            ps[:],
                        )
                    w2_s[m] = wt

                # ---- token tiles of this run ----
                for (col, T) in run_tiles:
                    ti = tile_idx
                    tile_idx += 1
                    if ti not in xio:
                        xio[ti] = load_x(xpool, col, T)
                    xcur = xio.pop(ti)
                    if ti + 1 < len(tiles):
                        nxt = tiles[ti + 1]
                        xio[ti + 1] = load_x(xpool, nxt[0], nxt[1])

                    hcs = []
                    for m in range(KH):
                        h_ps = hps_pool.tile([P, T], f32, name="hps", tag="hps")
                        for k in range(KI):
                            nc.tensor.matmul(
                                h_ps[:],
                                w1_s[k][:, m * P:(m + 1) * P],
                                xcur[:, k, :],
                                start=(k == 0), stop=(k == KI - 1),
                            )
                        hc = hpool.tile([P, T], bf16, name=f"hc{m}", tag="hc")
                        nc.scalar.activation(
                            hc[:], h_ps[:], AF.Gelu, bias=bias1_s[:, m:m + 1]
                        )
                        hcs.append(hc)

                    yc = ypool.tile([P, KO, T], bf16, name="yio", tag="yio")
                    for o in range(KO):
                        y_ps = yps_pool.tile([P, T], f32, name="yps", tag="yps")
                        for m in range(KH):
                            nc.tensor.matmul(
                                y_ps[:],
                                w2_s[m][:, o * P:(o + 1) * P],
                                hcs[m][:],
                                start=(m == 0), stop=(m == KH - 1),
                            )
                        nc.scalar.activation(
                            yc[:, o, :], y_ps[:], AF.Identity,
                            bias=bias2_s[:, o:o + 1],
                        )
                    nc.sync.dma_start(
                        yt_d[:, col:col + T].rearrange("(o p) c -> p o c", p=P),
                        yc[:],
                    )
    nc.compile()
    return nc


def _get_nc(tiles):
    nc = _nc_cache.get(tiles)
    if nc is None:
        nc = _nc_cache[tiles] = _build(tiles)
    return nc


def _run(inputs, trace=False):
    from concourse.bass_utils import run_bass_kernel_spmd

    x = np.asarray(inputs["x"], dtype=np.float32)
    w1 = np.asarray(inputs["w1"], dtype=np.float32)
    bias1 = np.asarray(inputs["bias1"], dtype=np.float32)
    a1 = np.asarray(inputs["a1"], dtype=np.float32)
    b1 = np.asarray(inputs["b1"], dtype=np.float32)
    w2 = np.asarray(inputs["w2"], dtype=np.float32)
    bias2 = np.asarray(inputs["bias2"], dtype=np.float32)
    a2 = np.asarray(inputs["a2"], dtype=np.float32)
    b2 = np.asarray(inputs["b2"], dtype=np.float32)
    chunk_sizes = tuple(int(v) for v in np.asarray(inputs["chunk_sizes"]))
    eids = tuple(int(v) for v in np.asarray(inputs["expert_indices"]))
    assert sum(chunk_sizes) == S

    tiles, groups = _plan_tiles(chunk_sizes, eids)
    nc = _get_nc(tiles)

    shared = {
        "w1t": np.ascontiguousarray(w1.T).astype(BF16),
        "a1t": a1.astype(BF16),
        "b1t": np.ascontiguousarray((SCALE * b1).transpose(0, 2, 1)).astype(BF16),
        "bias1": bias1,
        "w2t": np.ascontiguousarray(w2.T).astype(BF16),
        "a2t": a2.astype(BF16),
        "b2t": np.ascontiguousarray((SCALE * b2).transpose(0, 2, 1)).astype(BF16),
        "bias2": bias2,
    }

    in_maps = []
    for c in range(NCORES):
        xc = x[c * BPC:(c + 1) * BPC]                    # [BPC, S, IN]
        parts = [
            xc[:, s0:s0 + sz, :].reshape(BPC * sz, IN) for (s0, sz) in groups
        ]
        xt = np.concatenate(parts, axis=0).T             # [IN, TPC]
        m = dict(shared)
        m["xt"] = np.ascontiguousarray(xt).astype(BF16)
        in_maps.append(m)

    res = run_bass_kernel_spmd(
        nc, in_maps, core_ids=list(range(NCORES)), trace=trace
    )

    y = np.empty((B, S, OUT), dtype=np.float32)
    for c in range(NCORES):
        yt = np.asarray(res.results[c]["yt"]).astype(np.float32).T  # [TPC, OUT]
        col = 0
        for (s0, sz) in groups:
            gsz = BPC * sz
            y[c * BPC:(c + 1) * BPC, s0:s0 + sz, :] = (
                yt[col:col + gsz].reshape(BPC, sz, OUT)
            )
            col += gsz
    return y, res


def kernel(**inputs) -> np.ndarray:
    y, _ = _run(inputs, trace=False)
    return y
